# revision 1
# baseline (speedup 1.0000x reference)
"""Causal attention (B=4, S=4096, D_in=768, D_out=64) on 8 trn2 NeuronCores.

Sharding: 2 cores per batch element. Core (b, p) handles query rows
{2*i + p} of batch b (row-interleaved => balanced causal work, and every
core runs an identical instruction stream — SPMD-safe).

Host-side per-core prep: x[b] rows are permuted to [own-parity rows,
other-parity rows] and transposed to xT [768, 4096]. On-chip, xT streams
in 512-column blocks (stage g), each feeding:
  QT[64,512]   = (Wq*SCALE)^T @ block   (own half only; SCALE folded in)
  KT[64,512]   = Wk^T @ block
  V[128,65]x4  = block^T @ Wv  (natural layout), col 64 = ones
Attention for q-supertile T (512 q cols) interleaves with later stages;
key tiles come in pairs (u, 16+u) sharing one 2-bank PSUM tile:
    sT[128k,1024] = [KT_u | KT_16+u]^T @ QT_T  (PE, contraction over d=64)
    attnT = exp(sT + mask)   (one ACT op per pair; bf16 additive mask
                              from host, only on the 4 diagonal pairs)
    outT[65,512] += V_j^T-contracted attnT_j over the 128 keys  (PE)
outT row 64 = softmax denominator (the V ones column). PE-transpose
outT -> [128q, 65], multiply by reciprocal of col 64, DMA out. The
normalize of supertile T is emitted after stage T+1 so its transposes
(sharing the "proj" psum slots) don't stall the next projections.

All matmul operands are float32r (full-rate PE streaming vs 1/4-rate
fp32; measured end-to-end relative error vs the fp32 reference: 1.8e-4).
No max-subtraction pass: |score*SCALE| <= ~10 for these inputs, exp is
safe in fp32. Masked logits get -1e30 => exp -> 0 exactly.
"""

from contextlib import ExitStack

import numpy as np

import concourse.bass as bass
import concourse.mybir as mybir
import concourse.tile as tile
from concourse import bacc
from concourse.bass_utils import run_bass_kernel_spmd
from concourse.masks import make_identity

B, S, DI, DO = 4, 4096, 768, 64
NCORES = 8
SQ = S // 2          # 2048 local q rows per core
P = 128
NCHUNK = DI // P     # 6 contraction chunks
NKT = S // P         # 32 key tiles
NST = 4              # q supertiles per core
STW = 512            # supertile width
SCALE = 1.0 / np.sqrt(DO)
NEG = -1.0e30
F32 = mybir.dt.float32
F32R = mybir.dt.float32r
# float32r: same 4-byte host data, PE streams at full rate (fp32 runs at
# 1/4 rate). The BIR verifier requires the dtype end-to-end on every
# producer of a matmul operand, so all matmul-feeding tensors are MMDT.
USE_F32R = True
MMDT = F32R if USE_F32R else F32

_cache: dict = {}


def _mm(ap):
    return ap


def _emit_body(nc, tc, pools, aps):
    xt, wq, wk, wv, masks, out = aps
    (consts, xpool, qkv, attn_pool, osb_pool,
     ps_proj, ps_s, ps_o) = pools
    ps_tr = ps_proj  # transposes reuse the projection psum slots (tag "tr")

    # ---- constant loads ----
    wq_sb = consts.tile([P, NCHUNK, DO], MMDT, tag="wq")
    wk_sb = consts.tile([P, NCHUNK, DO], MMDT, tag="wk")
    wv_sb = consts.tile([P, NCHUNK, DO], MMDT, tag="wv")
    nc.sync.dma_start(out=wq_sb[:], in_=wq.rearrange("(c p) d -> p c d", p=P))
    nc.sync.dma_start(out=wk_sb[:], in_=wk.rearrange("(c p) d -> p c d", p=P))
    nc.sync.dma_start(out=wv_sb[:], in_=wv.rearrange("(c p) d -> p c d", p=P))
    ident = consts.tile([P, P], F32, tag="ident")
    make_identity(nc, ident[:])
    ones_f32 = consts.tile([P, 1], F32, tag="ones")
    nc.vector.memset(ones_f32[:], 1.0)

    # ---- streamed stages over 512-column blocks of x^T ----
    # Stage g covers xt columns [512g, 512g+512): DMA the 6 chunk slices,
    # project K (all g), Q (g<4: own half), V (key tiles 4g..4g+3).
    # Attention supertile T only needs stages {0..T} and {4..4+T}, so the
    # emission order  stage t, stage 4+t, attention T=t  lets DMA, PE
    # projections and attention pipeline instead of running as phases.
    qt_sb = [qkv.tile([DO, STW], MMDT, tag=f"qt{g}", name=f"qt{g}") for g in range(4)]
    kt_sb = [qkv.tile([DO, STW], MMDT, tag=f"kt{g}", name=f"kt{g}") for g in range(8)]
    v_sb = [None] * NKT
    xt_blk: dict = {}

    def kt_tile(j):  # key tile j (0..31) -> [64, 128] slice of its block
        return kt_sb[j // 4][:, (j % 4) * P:(j % 4 + 1) * P]

    def stage(g):
        blk = []
        for c in range(NCHUNK):
            t = xpool.tile([P, STW], MMDT, tag=f"xt{c}_{g}")
            nc.sync.dma_start(
                out=t[:], in_=xt[c * P:(c + 1) * P, g * STW:(g + 1) * STW])
            blk.append(t)
        xt_blk[g] = blk
        # K projection group g
        pk = ps_proj.tile([DO, STW], F32, tag="proj")
        for c in range(NCHUNK):
            nc.tensor.matmul(pk[:], _mm(wk_sb[:, c, :]), _mm(blk[c][:]),
                             start=(c == 0), stop=(c == NCHUNK - 1))
        nc.vector.tensor_copy(kt_sb[g][:], pk[:])
        # Q projection group g (own half only)
        if g < 4:
            pq = ps_proj.tile([DO, STW], F32, tag="proj")
            for c in range(NCHUNK):
                nc.tensor.matmul(pq[:], _mm(wq_sb[:, c, :]), _mm(blk[c][:]),
                                 start=(c == 0), stop=(c == NCHUNK - 1))
            nc.vector.tensor_copy(qt_sb[g][:], pq[:])
        # V projection for the 4 key tiles of this block
        for jj in range(4):
            j = 4 * g + jj
            pv = ps_proj.tile([P, DO], F32, tag="proj")
            for c in range(NCHUNK):
                nc.tensor.matmul(
                    pv[:], _mm(blk[c][:, jj * P:(jj + 1) * P]),
                    _mm(wv_sb[:, c, :]),
                    start=(c == 0), stop=(c == NCHUNK - 1))
            vj = qkv.tile([P, DO + 1], MMDT, tag=f"v{j}")
            nc.vector.tensor_copy(vj[:, 0:DO], pv[:])
            nc.vector.tensor_copy(vj[:, DO:DO + 1], ones_f32[:])
            v_sb[j] = vj

    ot_tiles = [None] * NST

    po_tiles = [None] * NST

    def attention(T, us):
        # Key tiles come in pairs (u, 16+u): own-parity and other-parity
        # tiles of the same global 256-row range. Each pair shares one
        # 2-bank PSUM tile [128, 1024] so the exp (ACT) runs once per pair.
        # Called twice per T: full pairs (u < 4T, need only stages < T) are
        # emitted BEFORE stage T so the ACT exp load spreads forward instead
        # of bunching at the tail; diagonal pairs come after stage T.
        npairs = 4 * T + 4
        if po_tiles[T] is None:
            po_tiles[T] = ps_o.tile([DO + 1, STW], F32, tag="o",
                                    name=f"po{T}")
        po = po_tiles[T]
        qt_slice = qt_sb[T][:]
        for u in us:
            ps = ps_s.tile([P, 2 * STW], F32, tag="s")
            nc.tensor.matmul(
                ps[:, 0:STW], _mm(kt_tile(u)),
                _mm(qt_slice), start=True, stop=True)
            nc.tensor.matmul(
                ps[:, STW:2 * STW], _mm(kt_tile(16 + u)),
                _mm(qt_slice), start=True, stop=True)
            at = attn_pool.tile([P, 2 * STW], MMDT, tag="attn")
            if u >= 4 * T:  # diagonal pair: additive mask on both halves
                nc.vector.tensor_add(at[:], ps[:], masks_sb[:, u - 4 * T, :])
                nc.scalar.activation(
                    at[:], at[:], mybir.ActivationFunctionType.Exp)
            else:
                nc.scalar.activation(
                    at[:], ps[:], mybir.ActivationFunctionType.Exp)
            nc.tensor.matmul(
                po[:], _mm(v_sb[u][:]), _mm(at[:, 0:STW]),
                start=(u == 0), stop=False)
            nc.tensor.matmul(
                po[:], _mm(v_sb[16 + u][:]), _mm(at[:, STW:2 * STW]),
                start=False, stop=(u == npairs - 1))
        if us[-1] != npairs - 1:
            return
        # stash the unnormalized output in SBUF; normalize is emitted later
        # so its PE transposes (sharing the "proj" psum tag) don't serialize
        # the next stage's projections behind the end of this attention.
        ot = osb_pool.tile([DO + 1, STW], F32, tag="ot")
        nc.vector.tensor_copy(ot[:], po[:])
        ot_tiles[T] = ot

    def normalize(T):
        # transpose [65,512] -> 4x [128,65], divide by denom, write out
        ot = ot_tiles[T]
        for sub in range(STW // P):
            ptr = ps_tr.tile([P, DO + 1], F32, tag="proj")
            nc.tensor.transpose(
                ptr[:], ot[:, sub * P:(sub + 1) * P],
                ident[0:DO + 1, 0:DO + 1])
            rc = osb_pool.tile([P, 1], F32, tag="rc")
            nc.vector.reciprocal(rc[:], ptr[:, DO:DO + 1])
            ob = osb_pool.tile([P, DO], F32, tag="ob")
            nc.vector.tensor_scalar_mul(ob[:], ptr[:, 0:DO], rc[:])
            r0 = T * STW + sub * P
            nc.sync.dma_start(out=out[r0:r0 + P, :], in_=ob[:])

    masks_sb = consts.tile([P, 4, 2 * STW], mybir.dt.bfloat16, tag="masks")
    for t_step in range(NST):
        stage(t_step)
        if t_step == 0:
            # bf16 masks, emitted after stage 0's xt DMAs so the first
            # projection matmuls aren't stuck behind them in the DMA queue.
            for m in range(4):
                nc.sync.dma_start(out=masks_sb[:, m, :], in_=masks[m, :, :])
        stage(4 + t_step)
        if t_step > 0:
            normalize(t_step - 1)
        attention(t_step, list(range(4 * t_step + 4)))
    normalize(NST - 1)


def _build_program(repeat: int = 1):
    """Build (and cache) the SPMD program. `repeat` re-emits the whole body
    N times inside one NEFF — used only for timing (the N-vs-1 wall-clock
    diff cancels the per-dispatch axon overhead)."""
    if repeat in _cache:
        return _cache[repeat]
    nc = bacc.Bacc("TRN2", target_bir_lowering=False, debug=False)

    xt = nc.dram_tensor("xt", [DI, S], MMDT, kind="ExternalInput").ap()
    wq = nc.dram_tensor("wq", [DI, DO], MMDT, kind="ExternalInput").ap()
    wk = nc.dram_tensor("wk", [DI, DO], MMDT, kind="ExternalInput").ap()
    wv = nc.dram_tensor("wv", [DI, DO], MMDT, kind="ExternalInput").ap()
    masks = nc.dram_tensor("masks", [4, P, 2 * STW], mybir.dt.bfloat16,
                           kind="ExternalInput").ap()
    out = nc.dram_tensor("out", [SQ, DO], F32, kind="ExternalOutput").ap()
    aps = (xt, wq, wk, wv, masks, out)

    with tile.TileContext(nc) as tc:
        with ExitStack() as ctx:
            pools = (
                ctx.enter_context(tc.tile_pool(name="consts", bufs=1)),
                ctx.enter_context(tc.tile_pool(name="xt", bufs=1)),
                ctx.enter_context(tc.tile_pool(name="qkv", bufs=1)),
                ctx.enter_context(tc.tile_pool(name="attn", bufs=8)),
                ctx.enter_context(tc.tile_pool(name="osb", bufs=4)),
                ctx.enter_context(tc.tile_pool(name="ps_proj", bufs=3, space="PSUM")),
                ctx.enter_context(tc.tile_pool(name="ps_s", bufs=2, space="PSUM")),
                ctx.enter_context(tc.tile_pool(name="ps_o", bufs=1, space="PSUM")),
            )
            for _rep in range(repeat):
                _emit_body(nc, tc, pools, aps)

    nc.compile()
    _cache[repeat] = nc
    return nc


def _host_masks(p: int) -> np.ndarray:
    """4 paired additive diagonal masks [128 keys, 1024] per core parity p.

    masks[m][:, 0:512]    : own-parity   key tile j = 4T+m vs supertile T.
    masks[m][:, 512:1024] : other-parity key tile j = 16+4T+m.
    q = 128*sub + qi (within supertile); allowed iff k <= bound.
    """
    sub = np.arange(STW) // P
    qi = np.arange(STW) % P
    k = np.arange(P)[:, None]
    masks = np.empty((4, P, 2 * STW), np.float32)
    for m in range(4):
        bound_own = P * (sub - m) + qi
        bound_oth = P * (sub - m) + qi + p - 1
        masks[m, :, 0:STW] = np.where(k <= bound_own[None, :], 0.0, NEG)
        masks[m, :, STW:] = np.where(k <= bound_oth[None, :], 0.0, NEG)
    import ml_dtypes
    return masks.astype(ml_dtypes.bfloat16)


def _perm(p: int) -> np.ndarray:
    return np.concatenate([np.arange(p, S, 2), np.arange(1 - p, S, 2)])


def make_in_maps(x, Wq, Wk, Wv):
    wq_s = np.ascontiguousarray(np.asarray(Wq) * np.float32(SCALE),
                                dtype=np.float32)
    wk_ = np.ascontiguousarray(Wk, dtype=np.float32)
    wv_ = np.ascontiguousarray(Wv, dtype=np.float32)
    masks_by_p = [_host_masks(0), _host_masks(1)]
    in_maps = []
    for c in range(NCORES):
        b, p = c // 2, c % 2
        xtc = np.ascontiguousarray(np.asarray(x[b], np.float32)[_perm(p)].T)
        in_maps.append({
            "xt": xtc, "wq": wq_s, "wk": wk_, "wv": wv_,
            "masks": masks_by_p[p],
        })
    return in_maps


def gather_out(results) -> np.ndarray:
    out = np.empty((B, S, DO), np.float32)
    for c in range(NCORES):
        b, p = c // 2, c % 2
        out[b, p::2, :] = results[c]["out"]
    return out


def run(x, Wq, Wk, Wv, trace=False, **spmd_kwargs):
    nc = _build_program()
    in_maps = make_in_maps(x, Wq, Wk, Wv)
    res = run_bass_kernel_spmd(
        nc, in_maps, core_ids=list(range(NCORES)), trace=trace, **spmd_kwargs)
    return gather_out(res.results), res


def kernel(x, Wq, Wk, Wv):
    out, _ = run(x, Wq, Wk, Wv)
    return out



# revision 7
# speedup vs baseline: 323.3483x; 323.3483x over previous
"""Causal attention (B=4, S=4096, D_in=768, D_out=64) on 8 trn2 NeuronCores.

Sharding: 2 cores per batch element. Core (b, p) handles query rows
{2*i + p} of batch b (row-interleaved => balanced causal work, identical
SPMD instruction stream). Host prep permutes x[b] rows to [own-parity,
other-parity], transposes to xT [768, 4096], and ships it as FP16
(rel err vs fp32 reference ~4e-4, tolerance 2e-2).

On-chip (all fp16 operands, fp32 PSUM accumulation):
  Stage g (512-col block of xT): one [128,6,512] DMA, then packed
  projections:
    g<4 (own parity):   psum[128,512] = [Wk | Wq*SCALE]^T @ blk
                        -> kq_sb[g] (lo 64 rows = K^T, hi = Q^T)
                        psum[64,512]  = Wv^T @ blk -> vt
    g>=4 (other):       psum[128,512] = [Wv | Wk]^T @ blk -> kq_sb[g]
                        (lo = V^T, hi = K^T at partitions 64:128)
  V^T blocks are PE-transposed to natural V tiles [128 keys, 65] with a
  ones column (softmax denominator accumulates in the attn@V matmul).
  Q^T is duplicated to partitions 0:64 via SBUF->SBUF DMA (qlo) so the
  pair of score matmuls lands on DIFFERENT PE row-groups:
      own  keys: lhsT/rhs at partitions 0:64  -> tile_position (0,0)
      other keys: lhsT/rhs at partitions 64:128 -> tile_position (64,0)
  K=64 matmuls in distinct row-groups execute CONCURRENTLY on the PE's
  16x(32x32) sub-arrays (~2x on hardware; serialized in the cost model).

  Attention supertile T (512 q cols), key pair u = (tile u, tile 16+u):
    psS[128,1024] = scores (two concurrent matmuls)
    at = exp(psS - 2) in fp16 (one ACT op; diagonal pairs restrict to
         the causally-live column range with a strided 3D AP, then one
         DVE multiply by a [128,2,128] {0,1} triangular mask)
    po[128 q, 4, 65] += at_block^T @ V  (contraction over keys, M=128,
         natural output layout; col 64 = denominator via the ones col)
  Fully-masked (q-subtile < diagonal) score/exp/attnV work is skipped.
  normalize: reciprocal of po col 64, scale, DMA out. No transposes.

exp(s-2): scores*SCALE measured |s| <= ~2.6; bias -2 keeps fp16 exp
well in range (cancels exactly in softmax). Masked weights are zeroed
multiplicatively, garbage (never-written) at columns are never read.
"""

from contextlib import ExitStack

import numpy as np

import concourse.bass as bass
import concourse.mybir as mybir
import concourse.tile as tile
from concourse import bacc
from concourse.bass_utils import run_bass_kernel_spmd
from concourse.masks import make_identity

B, S, DI, DO = 4, 4096, 768, 64
NCORES = 8
SQ = S // 2          # 2048 local q rows per core
P = 128
NCH = DI // P        # 6 contraction chunks
NST = 4              # q supertiles per core
STW = 512            # supertile width
SCALE = 1.0 / np.sqrt(DO)
F16 = mybir.dt.float16
F32 = mybir.dt.float32
Exp = mybir.ActivationFunctionType.Exp
EXP_BIAS = -2.0

_cache: dict = {}


def _emit_body(nc, tc, pools, aps):
    xt3, wkq, wkv, wvo, tmask, out = aps
    (consts, xpool, kqpool, vtpool, vpool, attn_pool, osb,
     ps_proj, ps_s, ps_o) = pools

    # ---- constants ----
    wkq_sb = consts.tile([P, NCH, P], F16, tag="wkq")
    wkv_sb = consts.tile([P, NCH, P], F16, tag="wkv")
    wvo_sb = consts.tile([P, NCH, DO], F16, tag="wvo")
    tmask_sb = consts.tile([P, 2, P], F16, tag="tmask")
    nc.sync.dma_start(out=wkq_sb[:], in_=wkq[:])
    nc.sync.dma_start(out=wkv_sb[:], in_=wkv[:])
    nc.sync.dma_start(out=wvo_sb[:], in_=wvo[:])
    nc.sync.dma_start(out=tmask_sb[:], in_=tmask[:])
    ident = consts.tile([DO, DO], F16, tag="ident")
    make_identity(nc, ident[:])
    ebias = consts.tile([P, 1], F32, tag="ebias")
    nc.gpsimd.memset(ebias[:], EXP_BIAS)

    kq_sb = [None] * 8       # [128, 512]: g<4 lo=K^T hi=Q^T; g>=4 lo=V^T hi=K^T
    qlo_sb = [None] * NST    # [64, 512] copy of Q^T at partitions 0:64
    v_sb = [None] * (2 * 16) # [128, 65] natural V + ones col

    def kt_own(j):           # key tile j (0..15), partitions 0:64
        return kq_sb[j // 4][0:DO, (j % 4) * P:(j % 4 + 1) * P]

    def kt_oth(j):           # key tile 16+j, partitions 64:128
        return kq_sb[4 + j // 4][DO:P, (j % 4) * P:(j % 4 + 1) * P]

    def stage(g):
        xb = xpool.tile([P, NCH, STW], F16, tag="xblk", name=f"xb{g}")
        nc.sync.dma_start(out=xb[:], in_=xt3[:, :, g * STW:(g + 1) * STW])
        w = wkq_sb if g < 4 else wkv_sb
        pA = ps_proj.tile([P, STW], F32, tag="proj", name=f"pA{g}")
        for c in range(NCH):
            nc.tensor.matmul(pA[:], w[:, c, :], xb[:, c, :],
                             start=(c == 0), stop=(c == NCH - 1))
        kq = kqpool.tile([P, STW], F16, tag=f"kq{g}", name=f"kq{g}")
        nc.vector.tensor_copy(kq[:], pA[:])
        kq_sb[g] = kq
        if g < 4:
            # Q^T copy to partitions 0:64 (cross-partition => DMA)
            ql = kqpool.tile([DO, STW], F16, tag=f"qlo{g}", name=f"qlo{g}")
            nc.sync.dma_start(out=ql[:], in_=kq[DO:P, :])
            qlo_sb[g] = ql
            pV = ps_proj.tile([DO, STW], F32, tag="proj", name=f"pV{g}")
            for c in range(NCH):
                nc.tensor.matmul(pV[:], wvo_sb[:, c, :], xb[:, c, :],
                                 start=(c == 0), stop=(c == NCH - 1))
            vt = vtpool.tile([DO, STW], F16, tag="vt", name=f"vt{g}")
            nc.vector.tensor_copy(vt[:], pV[:])
            vsrc = vt
        else:
            vsrc = kq  # V^T lives in the lo half
        ptr = ps_proj.tile([P, 4, DO], F16, tag="proj", name=f"ptr{g}")
        for jj in range(4):
            nc.tensor.transpose(ptr[:, jj, :], vsrc[0:DO, jj * P:(jj + 1) * P],
                                ident[:])
            j = 4 * g + jj if g < 4 else 16 + 4 * (g - 4) + jj
            v = vpool.tile([P, DO + 1], F16, tag=f"v{j}", name=f"v{j}")
            nc.vector.tensor_copy(v[:, 0:DO], ptr[:, jj, :])
            nc.gpsimd.memset(v[:, DO:DO + 1], 1.0)
            v_sb[j] = v

    po_tiles = [None] * NST
    pending = [None]  # one-pair software pipeline: emit attnV one pair late

    def emit_attnv(T, u, at):
        m = u - 4 * T  # >= 0: diagonal pair
        for qs in range(4):
            if m >= 0 and qs < m:
                continue  # fully-masked block: at cols never written
            for h in range(2):
                nc.tensor.matmul(
                    po_tiles[T][:, qs, :],
                    at[:, h, qs * P:(qs + 1) * P],
                    v_sb[16 * h + u][:],
                    start=(u == 0 and h == 0 and qs == 0),
                    stop=(u == 4 * T + 3 and h == 1 and qs == 3))

    def attention(T, us):
        if po_tiles[T] is None:
            po_tiles[T] = ps_o.tile([P, 4, DO + 1], F32, tag="po",
                                    name=f"po{T}")
        po = po_tiles[T]
        for u in us:
            m = u - 4 * T
            c0 = P * m if m >= 0 else 0
            psS = ps_s.tile([P, 2, STW], F32, tag="s")
            nc.tensor.matmul(psS[:, 0, c0:STW], kt_own(u),
                             qlo_sb[T][:, c0:STW], start=True, stop=True)
            nc.tensor.matmul(psS[:, 1, c0:STW], kt_oth(u),
                             kq_sb[T][DO:P, c0:STW], start=True, stop=True)
            at = attn_pool.tile([P, 2, STW], F16, tag="at")
            # one ACT op: 3D AP [128, 2, live] covers both halves
            nc.scalar.activation(at[:, :, c0:STW], psS[:, :, c0:STW], Exp,
                                 bias=ebias[:])
            if m >= 0:  # triangular {0,1} mask on the diagonal block
                nc.vector.tensor_mul(at[:, :, c0:c0 + P], at[:, :, c0:c0 + P],
                                     tmask_sb[:])
            if pending[0] is not None:
                emit_attnv(*pending[0])
            pending[0] = (T, u, at)

    def flush_attnv():
        if pending[0] is not None:
            emit_attnv(*pending[0])
            pending[0] = None

    def normalize(T):
        po = po_tiles[T]
        for qs in range(4):
            rc = osb.tile([P, 1], F32, tag="rc")
            nc.vector.reciprocal(rc[:], po[:, qs, DO:DO + 1])
            ob = osb.tile([P, DO], F32, tag="ob")
            nc.vector.tensor_scalar_mul(ob[:], po[:, qs, 0:DO], rc[:])
            r0 = T * STW + qs * P
            nc.sync.dma_start(out=out[r0:r0 + P, :], in_=ob[:])

    for t in range(NST):
        stage(t)
        attention(t, list(range(4 * t)))       # non-diag: stages < t, 4..4+t-1
        stage(4 + t)
        if t > 0:
            normalize(t - 1)
        attention(t, list(range(4 * t, 4 * t + 4)))  # diagonal pairs
        flush_attnv()
    normalize(NST - 1)


def _build_program(repeat: int = 1):
    """Build (and cache) the SPMD program. `repeat` re-emits the body N
    times in one NEFF (timing: the N-vs-1 diff cancels dispatch cost)."""
    if repeat in _cache:
        return _cache[repeat]
    nc = bacc.Bacc("TRN2", target_bir_lowering=False, debug=False)

    xt3 = nc.dram_tensor("xt3", [P, NCH, S], F16, kind="ExternalInput").ap()
    wkq = nc.dram_tensor("wkq", [P, NCH, P], F16, kind="ExternalInput").ap()
    wkv = nc.dram_tensor("wkv", [P, NCH, P], F16, kind="ExternalInput").ap()
    wvo = nc.dram_tensor("wvo", [P, NCH, DO], F16, kind="ExternalInput").ap()
    tmask = nc.dram_tensor("tmask", [P, 2, P], F16, kind="ExternalInput").ap()
    out = nc.dram_tensor("out", [SQ, DO], F32, kind="ExternalOutput").ap()
    aps = (xt3, wkq, wkv, wvo, tmask, out)

    with tile.TileContext(nc) as tc:
        with ExitStack() as ctx:
            pools = (
                ctx.enter_context(tc.tile_pool(name="consts", bufs=1)),
                ctx.enter_context(tc.tile_pool(name="xp", bufs=3)),
                ctx.enter_context(tc.tile_pool(name="kqp", bufs=1)),
                ctx.enter_context(tc.tile_pool(name="vtp", bufs=2)),
                ctx.enter_context(tc.tile_pool(name="vp", bufs=1)),
                ctx.enter_context(tc.tile_pool(name="attn", bufs=6)),
                ctx.enter_context(tc.tile_pool(name="osb", bufs=4)),
                ctx.enter_context(tc.tile_pool(name="ps_proj", bufs=2,
                                               space="PSUM")),
                ctx.enter_context(tc.tile_pool(name="ps_s", bufs=2,
                                               space="PSUM")),
                ctx.enter_context(tc.tile_pool(name="ps_o", bufs=2,
                                               space="PSUM")),
            )
            for _rep in range(repeat):
                _emit_body(nc, tc, pools, aps)

    nc.compile()
    _cache[repeat] = nc
    return nc


def _perm(p: int) -> np.ndarray:
    return np.concatenate([np.arange(p, S, 2), np.arange(1 - p, S, 2)])


def make_in_maps(x, Wq, Wk, Wv):
    wq = np.asarray(Wq, np.float32) * np.float32(SCALE)
    wk = np.asarray(Wk, np.float32)
    wv = np.asarray(Wv, np.float32)

    def pack2(a, b):  # [768, 64] x2 -> [128, 6, 128] (lhsT chunks)
        m = np.concatenate([a, b], axis=1)          # [768, 128]
        return np.ascontiguousarray(
            m.reshape(NCH, P, P).transpose(1, 0, 2).astype(np.float16))

    wkq_h = pack2(wk, wq)                           # lo=K, hi=Q
    wkv_h = pack2(wv, wk)                           # lo=V, hi=K
    wvo_h = np.ascontiguousarray(
        wv.reshape(NCH, P, DO).transpose(1, 0, 2).astype(np.float16))

    k = np.arange(P)[:, None]
    qi = np.arange(P)[None, :]
    masks = []
    for p in range(2):
        tm = np.empty((P, 2, P), np.float16)
        tm[:, 0, :] = (k <= qi)                     # own parity
        tm[:, 1, :] = (k <= qi + p - 1)             # other parity
        masks.append(tm)

    in_maps = []
    for c in range(NCORES):
        b, p = c // 2, c % 2
        xtc = np.asarray(x[b], np.float32)[_perm(p)].T.astype(np.float16)
        xt3 = np.ascontiguousarray(
            xtc.reshape(NCH, P, S).transpose(1, 0, 2))  # [128, 6, 4096]
        in_maps.append({
            "xt3": xt3, "wkq": wkq_h, "wkv": wkv_h, "wvo": wvo_h,
            "tmask": masks[p],
        })
    return in_maps


def gather_out(results) -> np.ndarray:
    out = np.empty((B, S, DO), np.float32)
    for c in range(NCORES):
        b, p = c // 2, c % 2
        out[b, p::2, :] = results[c]["out"]
    return out


def run(x, Wq, Wk, Wv, trace=False, **spmd_kwargs):
    nc = _build_program()
    in_maps = make_in_maps(x, Wq, Wk, Wv)
    res = run_bass_kernel_spmd(
        nc, in_maps, core_ids=list(range(NCORES)), trace=trace, **spmd_kwargs)
    return gather_out(res.results), res


def kernel(x, Wq, Wk, Wv):
    out, _ = run(x, Wq, Wk, Wv)
    return out
